# revision 32
# baseline (speedup 1.0000x reference)
"""Trainium2 Bass kernel for nn_Attend_62534723830373.

Reference computation (note: q is UNUSED by the reference):
    scores = einsum('bhid,bhjd->bhij', k, v) * (1/sqrt(128))
    scores = causal_mask(scores)            # strictly-upper masked
    attn   = softmax(scores, axis=-1)
    out    = einsum('bhij,bhjd->bhid', attn, v)

Shapes: [b=2, h=16, s=2048, d=128] fp32. b*h = 32 head-slices sharded
4-per-core across 8 NeuronCores (data/head parallel, no collectives).

Per-head dataflow on one core (matmul chain in bf16, fp32 accumulate):
  - SWDGE cast-load K, V (fp32 HBM -> bf16 SBUF, natural layout).
  - Build KT[d, s], VT[d, s] with one xbar DMA-transpose per matrix
    (out[p, bn, c] = in[c, bn*128 + p], verified on HW). Tile does not
    track the transpose's read of its SBUF input, so the load -> transpose
    RAW edge and the slot-reuse WAR edge are added explicitly.
  - For each 512-wide i-chunk, j-blocks processed in pairs sharing one
    1024-wide (2-bank) PSUM score tile and ONE exp instruction (halves
    the ACT per-instruction fixed cost):
      S^T[j, i] = (VT_blk).T @ KT_slice        (PE, contraction d)
      E = exp(SCALE * S^T)                     (ACT, PSUM -> SBUF bf16)
      diag block: E *= upper-tri 0/1 mask      (DVE)
      psum_o[i-blk] += E_slice.T @ [V_blk | 1] (PE, contraction j)
    The ones column makes column 128 of each accumulator the softmax
    denominator - numerator and denominator in one accumulation. Two
    accumulators share each PSUM bank; since matmul start=True clears
    has_written bank-wide, each bank is a single accumulation group
    (start only on the bank's first write, stop on its last; per-element
    has_written makes the second accumulator's first write an overwrite).
  - out = psum_o[:, 0:128] * (1 / psum_o[:, 128])  (DVE recip + mul)

kernel(**inputs) takes FULL unsharded inputs and returns the FULL output.
"""

import numpy as np

B, H, S, D = 2, 16, 2048, 128
N_CORES = 8
HPC = (B * H) // N_CORES  # heads per core = 4
NB = S // 128             # 16 j/i blocks per head
NCH = S // 512            # 4 i-chunks per head
SCALE = 0.08838834764831845

_CACHED_NC = None


def _build_nc():
    import concourse.bass as bass
    import concourse.mybir as mybir
    import concourse.tile as tile
    from concourse import bacc
    from concourse.masks import make_identity, make_upper_triangular
    from concourse.tile_rust import add_dep_helper
    from contextlib import ExitStack

    f32 = mybir.dt.float32
    bf16 = mybir.dt.bfloat16
    Exp = mybir.ActivationFunctionType.Exp

    nc = bacc.Bacc("TRN2", num_devices=N_CORES, debug=False)
    kd = nc.dram_tensor("k", [HPC, S, D], f32, kind="ExternalInput")
    vd = nc.dram_tensor("v", [HPC, S, D], f32, kind="ExternalInput")
    od = nc.dram_tensor("out", [HPC, S, D], f32, kind="ExternalOutput")

    with tile.TileContext(nc) as tc, ExitStack() as ctx:
        const = ctx.enter_context(tc.tile_pool(name="const", bufs=1))
        loadp = ctx.enter_context(tc.tile_pool(name="load", bufs=2))
        ktp = ctx.enter_context(tc.tile_pool(name="kt", bufs=2))
        expp = ctx.enter_context(tc.tile_pool(name="expp", bufs=4))
        outp = ctx.enter_context(tc.tile_pool(name="outp", bufs=2))
        smallp = ctx.enter_context(tc.tile_pool(name="small", bufs=4))
        ps_pool = ctx.enter_context(tc.tile_pool(name="ps", bufs=2, space="PSUM"))
        pt_pool = ctx.enter_context(tc.tile_pool(name="pt", bufs=2, space="PSUM"))
        po_pool = ctx.enter_context(tc.tile_pool(name="po", bufs=2, space="PSUM"))

        trimask_f32 = const.tile([128, 128], f32, tag="trimask_f32")
        make_upper_triangular(nc, trimask_f32[:, :], val=1.0, diag=True)
        trimask = const.tile([128, 128], bf16, tag="trimask")
        nc.vector.tensor_copy(trimask[:, :], trimask_f32[:, :])
        onesf32 = const.tile([128, NB], f32, tag="onesf32")
        nc.gpsimd.memset(onesf32[:, :], 1.0)
        identbf = const.tile([128, 128], bf16, tag="identbf")
        make_identity(nc, identbf[:, :])
        tr_hist = {}       # h -> (tr_k, tr_v) DMA-transpose handles
        mm1_last = {}      # h -> last MM1 instruction of head h

        prev_tr = None
        for h in range(HPC):
            # ---- loads: fp32 HBM -> bf16 SBUF (SWDGE cast), natural ----
            knat = loadp.tile([128, NB, 128], bf16, tag="knat")
            vnat = loadp.tile([128, NB, 128], bf16, tag="vnat")
            vones = loadp.tile([128, NB, 129], bf16, tag="vones")
            KT3 = ktp.tile([128, NB, 128], bf16, tag="KT")
            VT3 = ktp.tile([128, NB, 128], bf16, tag="VT")
            kview = kd.ap()[h].rearrange("(n p) d -> p n d", p=128)
            vview = vd.ap()[h].rearrange("(n p) d -> p n d", p=128)
            if h == 0:
                # head 0 gates kernel startup: load in 4-block chunks so the
                # first transposes/matmuls only wait on the first chunk
                for c0, c1 in ((0, 2), (2, 4), (4, 8), (8, 12), (12, 16)):
                    sl = slice(c0, c1)
                    nc.gpsimd.dma_start(knat[:, sl, :], kview[:, sl, :])
                    nc.gpsimd.dma_start(vnat[:, sl, :], vview[:, sl, :])
                # chunk the vones copies too so the first MM2s only wait
                # on the first chunk of V
                for c0, c1 in ((0, 2), (2, 4), (4, 8), (8, 12), (12, 16)):
                    sl = slice(c0, c1)
                    nc.sync.dma_start(vones[:, sl, 0:128], vnat[:, sl, :])
                    nc.vector.tensor_copy(vones[:, sl, 128], onesf32[:, sl])
            else:
                nc.gpsimd.dma_start(knat[:, :, :], kview[:, :, :])
                nc.gpsimd.dma_start(vnat[:, :, :], vview[:, :, :])
                # [V | ones] in bf16, 129 cols per j-block (the second matmul
                # runs in bf16; the denominator comes from the same bf16
                # weights so quantization cancels in the normalization)
                nc.sync.dma_start(vones[:, :, 0:128], vnat[:, :, :])
                nc.vector.tensor_copy(vones[:, :, 128], onesf32[:, :])
            # KT[d, s], VT[d, s] via PE transposes (bf16, 1 cycle/row) +
            # DVE copies - every op here is tracked by Tile, no manual deps
            for bn in range(NB):
                pstk = pt_pool.tile([128, 128], bf16, tag="pt", name=f"pstk_{h}_{bn}")
                nc.tensor.transpose(pstk[:, :], knat[:, bn, :], identbf[:, :])
                nc.vector.tensor_copy(KT3[:, bn, :], pstk[:, :])
                pstv = pt_pool.tile([128, 128], bf16, tag="pt", name=f"pstv_{h}_{bn}")
                nc.tensor.transpose(pstv[:, :], vnat[:, bn, :], identbf[:, :])
                nc.vector.tensor_copy(VT3[:, bn, :], pstv[:, :])
            KT = KT3.rearrange("p n d -> p (n d)")
            VT = VT3.rearrange("p n d -> p (n d)")

            out_sb = outp.tile([128, NB, 128], f32, tag="out_sb")

            # ---- main causal attention loop ----
            for ci in range(NCH):
                i0b = 4 * ci              # first i-block of chunk
                iend = (i0b + 4) * 128
                po = [
                    po_pool.tile([128, 258], f32, tag="po", name=f"po_{h}_{ci}_{u}")
                    for u in range(2)
                ]

                def po_ap(bi):
                    u = bi - i0b
                    return po[u // 2][:, (u % 2) * 129 : (u % 2) * 129 + 129]

                # pairs are emitted with one-pair lookahead: pair k+1's
                # score matmuls + exp come before pair k's MM2s, so the PE
                # has work while the first MM2 of a chunk waits for the po
                # banks to be freed by the previous chunk's epilogue
                pending = None  # (bj_pair_state, ex) awaiting MM2 emission
                pairs = list(range(0, i0b + 4, 2)) + [None]
                for bja in pairs:
                    cur = None
                    if bja is not None:
                        bjb = bja + 1
                        ista = max(i0b, bja) * 128
                        istb_ = max(i0b, bjb) * 128
                        n1a = iend - ista
                        n1b = iend - istb_
                        ps = ps_pool.tile([128, 1024], f32, tag="ps")
                        nc.tensor.matmul(
                            ps[:, 0:n1a],
                            VT[:, bja * 128 : (bja + 1) * 128],
                            KT[:, ista:iend],
                            start=True,
                            stop=True,
                        )
                        mm1_last[h] = nc.tensor.matmul(
                            ps[:, n1a : n1a + n1b],
                            VT[:, bjb * 128 : (bjb + 1) * 128],
                            KT[:, istb_:iend],
                            start=True,
                            stop=True,
                        )
                        ex = expp.tile([128, 1024], bf16, tag="ex")
                        nc.scalar.activation(
                            ex[:, 0 : n1a + n1b],
                            ps[:, 0 : n1a + n1b],
                            Exp,
                            scale=SCALE,
                        )
                        if bja >= i0b:
                            # diagonal blocks: zero j > i strict lower
                            # triangle (on the otherwise-idle GpSimd engine
                            # to keep DVE free for the transpose copies)
                            nc.gpsimd.tensor_mul(
                                ex[:, 0:128], ex[:, 0:128], trimask[:, :]
                            )
                        if bjb >= i0b:
                            nc.gpsimd.tensor_mul(
                                ex[:, n1a : n1a + 128],
                                ex[:, n1a : n1a + 128],
                                trimask[:, :],
                            )
                        cur = ((bja, ista, 0), (bjb, istb_, n1a), ex)
                    if pending is not None:
                        (pa, pb, pex) = pending
                        for bj, ist, off in (pa, pb):
                            for bi in range(ist // 128, i0b + 4):
                                c0 = off + bi * 128 - ist
                                nc.tensor.matmul(
                                    po_ap(bi),
                                    pex[:, c0 : c0 + 128],
                                    vones[:, bj, :],
                                    start=(bj == 0 and (bi - i0b) % 2 == 0),
                                    stop=(bj == bi and (bi - i0b) % 2 == 1),
                                    skip_group_check=True,
                                )
                    pending = cur
                for u in range(4):
                    bi = i0b + u
                    rc = smallp.tile([128, 1], f32, tag="rc")
                    nc.vector.reciprocal(rc[:, :], po_ap(bi)[:, 128:129])
                    nc.vector.tensor_scalar_mul(
                        out_sb[:, bi, :], po_ap(bi)[:, 0:128], rc[:, :]
                    )
                nc.sync.dma_start(
                    od.ap()[h].rearrange("(n p) d -> p n d", p=128)[
                        :, i0b : i0b + 4, :
                    ],
                    out_sb[:, i0b : i0b + 4, :],
                )

    nc.finalize()
    return nc


def _get_nc():
    global _CACHED_NC
    if _CACHED_NC is None:
        _CACHED_NC = _build_nc()
    return _CACHED_NC


def run_sharded(k, v, trace=False):
    """k, v: [B*H, S, D] fp32. Returns (out [B*H, S, D], BassKernelResults)."""
    from concourse import bass_utils

    nc = _get_nc()
    in_maps = [
        {
            "k": np.ascontiguousarray(k[c * HPC : (c + 1) * HPC]),
            "v": np.ascontiguousarray(v[c * HPC : (c + 1) * HPC]),
        }
        for c in range(N_CORES)
    ]
    res = bass_utils.run_bass_kernel_spmd(
        nc, in_maps, core_ids=list(range(N_CORES)), trace=trace
    )
    out = np.concatenate([res.results[c]["out"] for c in range(N_CORES)], axis=0)
    return out, res


def kernel(q, k, v):
    k = np.asarray(k, dtype=np.float32).reshape(B * H, S, D)
    v = np.asarray(v, dtype=np.float32).reshape(B * H, S, D)
    out, _ = run_sharded(k, v, trace=False)
    return out.reshape(B, H, S, D)


# revision 33
# speedup vs baseline: 1.1276x; 1.1276x over previous
"""Trainium2 Bass kernel for nn_Attend_62534723830373.

Reference computation (note: q is UNUSED by the reference):
    scores = einsum('bhid,bhjd->bhij', k, v) * (1/sqrt(128))
    scores = causal_mask(scores)            # strictly-upper masked
    attn   = softmax(scores, axis=-1)
    out    = einsum('bhij,bhjd->bhid', attn, v)

Shapes: [b=2, h=16, s=2048, d=128] fp32. b*h = 32 head-slices sharded
4-per-core across 8 NeuronCores (data/head parallel, no collectives).

Per-head dataflow on one core (matmul chain in bf16, fp32 accumulate):
  - SWDGE cast-load K, V (fp32 HBM -> bf16 SBUF, natural layout).
  - Build KT[d, s], VT[d, s] with one xbar DMA-transpose per matrix
    (out[p, bn, c] = in[c, bn*128 + p], verified on HW). Tile does not
    track the transpose's read of its SBUF input, so the load -> transpose
    RAW edge and the slot-reuse WAR edge are added explicitly.
  - For each 512-wide i-chunk, j-blocks processed in pairs sharing one
    1024-wide (2-bank) PSUM score tile and ONE exp instruction (halves
    the ACT per-instruction fixed cost):
      S^T[j, i] = (VT_blk).T @ KT_slice        (PE, contraction d)
      E = exp(SCALE * S^T)                     (ACT, PSUM -> SBUF bf16)
      diag block: E *= upper-tri 0/1 mask      (DVE)
      psum_o[i-blk] += E_slice.T @ [V_blk | 1] (PE, contraction j)
    The ones column makes column 128 of each accumulator the softmax
    denominator - numerator and denominator in one accumulation. Two
    accumulators share each PSUM bank; since matmul start=True clears
    has_written bank-wide, each bank is a single accumulation group
    (start only on the bank's first write, stop on its last; per-element
    has_written makes the second accumulator's first write an overwrite).
  - out = psum_o[:, 0:128] * (1 / psum_o[:, 128])  (DVE recip + mul)

kernel(**inputs) takes FULL unsharded inputs and returns the FULL output.
"""

import numpy as np

B, H, S, D = 2, 16, 2048, 128
N_CORES = 8
HPC = (B * H) // N_CORES  # heads per core = 4
NB = S // 128             # 16 j/i blocks per head
NCH = S // 512            # 4 i-chunks per head
SCALE = 0.08838834764831845

_CACHED_NC = None


def _build_nc():
    import concourse.bass as bass
    import concourse.mybir as mybir
    import concourse.tile as tile
    from concourse import bacc
    from concourse.masks import make_identity, make_upper_triangular
    from concourse.tile_rust import add_dep_helper
    from contextlib import ExitStack

    f32 = mybir.dt.float32
    bf16 = mybir.dt.bfloat16
    Exp = mybir.ActivationFunctionType.Exp

    nc = bacc.Bacc("TRN2", num_devices=N_CORES, debug=False)
    kd = nc.dram_tensor("k", [HPC, S, D], f32, kind="ExternalInput")
    vd = nc.dram_tensor("v", [HPC, S, D], f32, kind="ExternalInput")
    od = nc.dram_tensor("out", [HPC, S, D], f32, kind="ExternalOutput")

    with tile.TileContext(nc) as tc, ExitStack() as ctx:
        const = ctx.enter_context(tc.tile_pool(name="const", bufs=1))
        loadp = ctx.enter_context(tc.tile_pool(name="load", bufs=2))
        ktp = ctx.enter_context(tc.tile_pool(name="kt", bufs=2))
        expp = ctx.enter_context(tc.tile_pool(name="expp", bufs=4))
        outp = ctx.enter_context(tc.tile_pool(name="outp", bufs=2))
        smallp = ctx.enter_context(tc.tile_pool(name="small", bufs=4))
        ps_pool = ctx.enter_context(tc.tile_pool(name="ps", bufs=2, space="PSUM"))
        pt_pool = ctx.enter_context(tc.tile_pool(name="pt", bufs=2, space="PSUM"))
        po_pool = ctx.enter_context(tc.tile_pool(name="po", bufs=2, space="PSUM"))

        trimask_f32 = const.tile([128, 128], f32, tag="trimask_f32")
        make_upper_triangular(nc, trimask_f32[:, :], val=1.0, diag=True)
        trimask = const.tile([128, 128], bf16, tag="trimask")
        nc.vector.tensor_copy(trimask[:, :], trimask_f32[:, :])
        onesf32 = const.tile([128, NB], f32, tag="onesf32")
        nc.gpsimd.memset(onesf32[:, :], 1.0)
        identbf = const.tile([128, 128], bf16, tag="identbf")
        make_identity(nc, identbf[:, :])
        tr_hist = {}       # h -> (tr_k, tr_v) DMA-transpose handles
        mm1_last = {}      # h -> last MM1 instruction of head h

        prev_tr = None
        for h in range(HPC):
            # ---- loads: fp32 HBM -> bf16 SBUF (SWDGE cast), natural ----
            knat = loadp.tile([128, NB, 128], bf16, tag="knat")
            vnat = loadp.tile([128, NB, 128], bf16, tag="vnat")
            vones = loadp.tile([128, NB, 129], bf16, tag="vones")
            KT3 = ktp.tile([128, NB, 128], bf16, tag="KT")
            VT3 = ktp.tile([128, NB, 128], bf16, tag="VT")
            kview = kd.ap()[h].rearrange("(n p) d -> p n d", p=128)
            vview = vd.ap()[h].rearrange("(n p) d -> p n d", p=128)
            if h == 0:
                # head 0 gates kernel startup: load in 4-block chunks so the
                # first transposes/matmuls only wait on the first chunk
                for c0, c1 in ((0, 2), (2, 4), (4, 8), (8, 12), (12, 16)):
                    sl = slice(c0, c1)
                    nc.gpsimd.dma_start(knat[:, sl, :], kview[:, sl, :])
                    nc.gpsimd.dma_start(vnat[:, sl, :], vview[:, sl, :])
                # chunk the vones copies too so the first MM2s only wait
                # on the first chunk of V
                for c0, c1 in ((0, 2), (2, 4), (4, 8), (8, 12), (12, 16)):
                    sl = slice(c0, c1)
                    nc.vector.tensor_copy(vones[:, sl, 0:128], vnat[:, sl, :])
                    nc.vector.tensor_copy(vones[:, sl, 128], onesf32[:, sl])
            else:
                nc.gpsimd.dma_start(knat[:, :, :], kview[:, :, :])
                nc.gpsimd.dma_start(vnat[:, :, :], vview[:, :, :])
                # [V | ones] in bf16, 129 cols per j-block (the second matmul
                # runs in bf16; the denominator comes from the same bf16
                # weights so quantization cancels in the normalization)
                nc.vector.tensor_copy(vones[:, :, 0:128], vnat[:, :, :])
                nc.vector.tensor_copy(vones[:, :, 128], onesf32[:, :])
            # KT[d, s], VT[d, s] via PE transposes (bf16, 1 cycle/row) +
            # DVE copies - every op here is tracked by Tile, no manual deps
            for bn in range(NB):
                pstk = pt_pool.tile([128, 128], bf16, tag="pt", name=f"pstk_{h}_{bn}")
                nc.tensor.transpose(pstk[:, :], knat[:, bn, :], identbf[:, :])
                nc.vector.tensor_copy(KT3[:, bn, :], pstk[:, :])
                pstv = pt_pool.tile([128, 128], bf16, tag="pt", name=f"pstv_{h}_{bn}")
                nc.tensor.transpose(pstv[:, :], vnat[:, bn, :], identbf[:, :])
                nc.vector.tensor_copy(VT3[:, bn, :], pstv[:, :])
            KT = KT3.rearrange("p n d -> p (n d)")
            VT = VT3.rearrange("p n d -> p (n d)")

            out_sb = outp.tile([128, NB, 128], f32, tag="out_sb")

            # ---- main causal attention loop ----
            for ci in range(NCH):
                i0b = 4 * ci              # first i-block of chunk
                iend = (i0b + 4) * 128
                po = [
                    po_pool.tile([128, 258], f32, tag="po", name=f"po_{h}_{ci}_{u}")
                    for u in range(2)
                ]

                def po_ap(bi):
                    u = bi - i0b
                    return po[u // 2][:, (u % 2) * 129 : (u % 2) * 129 + 129]

                # pairs are emitted with one-pair lookahead: pair k+1's
                # score matmuls + exp come before pair k's MM2s, so the PE
                # has work while the first MM2 of a chunk waits for the po
                # banks to be freed by the previous chunk's epilogue
                pending = None  # (bj_pair_state, ex) awaiting MM2 emission
                pairs = list(range(0, i0b + 4, 2)) + [None]
                for bja in pairs:
                    cur = None
                    if bja is not None:
                        bjb = bja + 1
                        ista = max(i0b, bja) * 128
                        istb_ = max(i0b, bjb) * 128
                        n1a = iend - ista
                        n1b = iend - istb_
                        ps = ps_pool.tile([128, 1024], f32, tag="ps")
                        nc.tensor.matmul(
                            ps[:, 0:n1a],
                            VT[:, bja * 128 : (bja + 1) * 128],
                            KT[:, ista:iend],
                            start=True,
                            stop=True,
                        )
                        mm1_last[h] = nc.tensor.matmul(
                            ps[:, n1a : n1a + n1b],
                            VT[:, bjb * 128 : (bjb + 1) * 128],
                            KT[:, istb_:iend],
                            start=True,
                            stop=True,
                        )
                        ex = expp.tile([128, 1024], bf16, tag="ex")
                        nc.scalar.activation(
                            ex[:, 0 : n1a + n1b],
                            ps[:, 0 : n1a + n1b],
                            Exp,
                            scale=SCALE,
                        )
                        if bja >= i0b:
                            # diagonal blocks: zero j > i strict lower triangle
                            nc.vector.tensor_mul(
                                ex[:, 0:128], ex[:, 0:128], trimask[:, :]
                            )
                        if bjb >= i0b:
                            nc.vector.tensor_mul(
                                ex[:, n1a : n1a + 128],
                                ex[:, n1a : n1a + 128],
                                trimask[:, :],
                            )
                        cur = ((bja, ista, 0), (bjb, istb_, n1a), ex)
                    if pending is not None:
                        (pa, pb, pex) = pending
                        for bj, ist, off in (pa, pb):
                            for bi in range(ist // 128, i0b + 4):
                                c0 = off + bi * 128 - ist
                                nc.tensor.matmul(
                                    po_ap(bi),
                                    pex[:, c0 : c0 + 128],
                                    vones[:, bj, :],
                                    start=(bj == 0 and (bi - i0b) % 2 == 0),
                                    stop=(bj == bi and (bi - i0b) % 2 == 1),
                                    skip_group_check=True,
                                )
                    pending = cur
                for u in range(4):
                    bi = i0b + u
                    rc = smallp.tile([128, 1], f32, tag="rc")
                    nc.vector.reciprocal(rc[:, :], po_ap(bi)[:, 128:129])
                    nc.vector.tensor_scalar_mul(
                        out_sb[:, bi, :], po_ap(bi)[:, 0:128], rc[:, :]
                    )
                nc.sync.dma_start(
                    od.ap()[h].rearrange("(n p) d -> p n d", p=128)[
                        :, i0b : i0b + 4, :
                    ],
                    out_sb[:, i0b : i0b + 4, :],
                )

    nc.finalize()
    return nc


def _get_nc():
    global _CACHED_NC
    if _CACHED_NC is None:
        _CACHED_NC = _build_nc()
    return _CACHED_NC


def run_sharded(k, v, trace=False):
    """k, v: [B*H, S, D] fp32. Returns (out [B*H, S, D], BassKernelResults)."""
    from concourse import bass_utils

    nc = _get_nc()
    in_maps = [
        {
            "k": np.ascontiguousarray(k[c * HPC : (c + 1) * HPC]),
            "v": np.ascontiguousarray(v[c * HPC : (c + 1) * HPC]),
        }
        for c in range(N_CORES)
    ]
    res = bass_utils.run_bass_kernel_spmd(
        nc, in_maps, core_ids=list(range(N_CORES)), trace=trace
    )
    out = np.concatenate([res.results[c]["out"] for c in range(N_CORES)], axis=0)
    return out, res


def kernel(q, k, v):
    k = np.asarray(k, dtype=np.float32).reshape(B * H, S, D)
    v = np.asarray(v, dtype=np.float32).reshape(B * H, S, D)
    out, _ = run_sharded(k, v, trace=False)
    return out.reshape(B, H, S, D)
